# revision 8
# baseline (speedup 1.0000x reference)
"""Trainium2 Bass kernel for ActiveSpline (centripetal Catmull-Rom spline sampling).

Full input:  cps [16384, 16, 2] f32
Full output: pts [16384, 1024, 2] f32

Math: for each batch, build the closed-curve auxiliary control points and
centripetal knot *differences* (only knot diffs appear in the Barry-Goldman
pyramid), reduce each segment's 64-point evaluation to a cubic polynomial in
the normalized parameter u in [0,1]:

    pts(u) = g0 + g1 u + g2 u^2 + g3 u^3        (per batch, segment, coord)

The g-coefficients are computed on the Vector engine (batch-major layout:
partition p holds batches 16p..16p+15), then expanded to the 64 sample points
with TensorEngine matmuls against a constant block-Vandermonde matrix:

    out[b, (s,p,d)] = sum_{(s,d,k)} coefT[(s,d,k), b] * W[(s,d,k), (s,p,d)]

Sharding: purely batch-parallel, 2048 batches per core on 8 cores.
"""

import numpy as np

import concourse.bass as bass
import concourse.bacc as bacc
import concourse.mybir as mybir
import concourse.tile as tile
from concourse.bass_utils import run_bass_kernel_spmd

F32 = mybir.dt.float32
AF = mybir.ActivationFunctionType

N_CORES = 8
B_FULL = 16384
S = 16            # control points / segments (closed curve)
D = 2
P = 64            # samples per segment
BC = B_FULL // N_CORES   # 2048 batches per core
J = 16            # batches per partition (BC = 128 * J)
EPS = 1e-7
OUTW = S * P * D  # 2048 output floats per batch

# "rowtile": 4 concurrent K=32 matmuls via tile_position (4x PE throughput for fp32)
# "fullk":   4 plain K=128 matmuls (safe fallback)
MATMUL_MODE = "rowtile"


def _build_w() -> np.ndarray:
    """W [128, 2048]: block-diagonal expansion matrix.

    Row r = 32*c + rr encodes (s = 4*c + (rr>>3), d = (rr>>2)&1, k = rr&3).
    Col n = 512*c' + nn encodes (s' = 4*c' + nn//128, p = (nn%128)//2, d' = nn&1).
    W[r, n] = (s==s' and d==d') * u_p^k.
    The diagonal 32x512 blocks are identical, so rowtile mode can slice
    W[32c:32c+32, 512c:512(c+1)] and fullk mode can slice W[:, 512c:512(c+1)].
    """
    f = np.float32
    u = (np.arange(P, dtype=f) / f(P - 1)).astype(f)
    pow_u = np.stack([np.ones(P, f), u, u * u, (u * u) * u])  # [4, 64]
    w4 = np.zeros((32, 512), f)
    for rr in range(32):
        sl, d, k = rr >> 3, (rr >> 2) & 1, rr & 3
        w4[rr, sl * 128 + np.arange(P) * 2 + d] = pow_u[k]
    w = np.zeros((128, 2048), f)
    for c in range(4):
        w[32 * c:32 * c + 32, 512 * c:512 * (c + 1)] = w4
    return w


def _build_nc(n_reps: int = 1):
    nc = bacc.Bacc("TRN2", target_bir_lowering=False, debug=False,
                   enable_asserts=False, num_devices=N_CORES)

    cps_d = nc.dram_tensor("cps", [BC, S * D], F32, kind="ExternalInput")
    w_d = nc.dram_tensor("wmat", [128, OUTW], F32, kind="ExternalInput")
    id_d = nc.dram_tensor("ident", [128, 128], F32, kind="ExternalInput")
    out_d = nc.dram_tensor("out", [BC, OUTW], F32, kind="ExternalOutput")

    with tile.TileContext(nc) as tc:
        with (
            tc.tile_pool(name="const", bufs=1) as const,
            tc.tile_pool(name="inp", bufs=2) as inp,
            tc.tile_pool(name="work", bufs=2) as work,
            tc.tile_pool(name="lhs", bufs=16) as lhsp,
            tc.tile_pool(name="osb", bufs=2) as osbp,
            tc.tile_pool(name="pst", bufs=2, space="PSUM") as pst,
            tc.tile_pool(name="ps0", bufs=1, space="PSUM") as ps0p,
            tc.tile_pool(name="ps1", bufs=1, space="PSUM") as ps1p,
        ):
            # ---- constants ----
            w_sb = const.tile([128, OUTW], F32)
            i_sb = const.tile([128, 128], F32)
            nc.sync.dma_start(w_sb[:], w_d.ap())
            nc.sync.dma_start(i_sb[:], id_d.ap())

            eps_t = const.tile([128, 1], F32)
            zero_t = const.tile([128, 1], F32)
            nc.vector.memset(eps_t[:], float(EPS))
            nc.vector.memset(zero_t[:], 0.0)

            for _rep in range(n_reps):
                _emit_once(nc, tc, inp, work, lhsp, osbp, pst, ps0p, ps1p,
                           cps_d, out_d, w_sb, i_sb, eps_t, zero_t)

    nc.compile()
    return nc


def _emit_once(nc, tc, inp, work, lhsp, osbp, pst, ps0p, ps1p,
               cps_d, out_d, w_sb, i_sb, eps_t, zero_t):
    if True:
        if True:
            # ---- input ----
            x = inp.tile([128, J * S * D], F32)      # [p, (j, s, d)]
            nc.sync.dma_start(
                x[:].rearrange("p (j q) -> p j q", j=J),
                cps_d.ap().rearrange("(p j) q -> p j q", j=J),
            )

            xv = x[:].rearrange("p (j s d) -> p j s d", j=J, s=S, d=D)

            # ---- phase B: knots + cubic coefficients (vector engine) ----
            diff = work.tile([128, J * 18 * 2], F32)   # aux diffs, i = 0..17
            sq = work.tile([128, J * 18 * 2], F32)
            ss = work.tile([128, J * 18], F32)         # squared seg lengths
            sqt = work.tile([128, J * 18], F32)
            sl = work.tile([128, J * 18], F32)         # seg_len = ss^0.25
            rsl = work.tile([128, J * 18], F32)
            coef = work.tile([128, J * 128], F32)      # [p, (j, s, d, k)]

            dv = diff[:].rearrange("p (j i d) -> p j i d", j=J, i=18, d=D)
            qv = sq[:].rearrange("p (j i d) -> p j i d", j=J, i=18, d=D)
            ssv = ss[:].rearrange("p (j i) -> p j i", j=J)
            slv = sl[:].rearrange("p (j i) -> p j i", j=J)
            rslv = rsl[:].rearrange("p (j i) -> p j i", j=J)
            cfv = coef[:].rearrange("p (j s d k) -> p j s d k", j=J, s=S, d=D, k=4)

            vec = nc.vector
            # inner aux diffs: D[i] = cps[i] - cps[i-1] (i=1..15), D[16] = cps0 - cps15
            vec.tensor_sub(dv[:, :, 1:16, :], xv[:, :, 1:16, :], xv[:, :, 0:15, :])
            vec.tensor_sub(dv[:, :, 16, :], xv[:, :, 0, :], xv[:, :, 15, :])
            vec.tensor_mul(qv[:, :, 1:17, :], dv[:, :, 1:17, :], dv[:, :, 1:17, :])
            vec.tensor_add(ssv[:, :, 1:17], qv[:, :, 1:17, 0], qv[:, :, 1:17, 1])

            l01 = work.tile([128, J], F32)
            llast = work.tile([128, J], F32)
            rl01 = work.tile([128, J], F32)
            rllast = work.tile([128, J], F32)
            r1 = work.tile([128, J], F32)
            r2 = work.tile([128, J], F32)
            # l01/llast include the +EPS inside the sqrt (as the reference does)
            nc.scalar.activation(l01[:], ssv[:, :, 1], AF.Sqrt, bias=eps_t[:])
            nc.scalar.activation(llast[:], ssv[:, :, 16], AF.Sqrt, bias=eps_t[:])
            vec.reciprocal(rl01[:], l01[:])
            vec.reciprocal(rllast[:], llast[:])
            vec.tensor_mul(r1[:], l01[:], rllast[:])     # l01/llast
            vec.tensor_mul(r2[:], llast[:], rl01[:])     # llast/l01
            for d in range(D):
                vec.tensor_mul(dv[:, :, 0, d], r1[:], dv[:, :, 16, d])
                vec.tensor_mul(dv[:, :, 17, d], r2[:], dv[:, :, 1, d])
            vec.tensor_mul(qv[:, :, 0, :], dv[:, :, 0, :], dv[:, :, 0, :])
            vec.tensor_mul(qv[:, :, 17, :], dv[:, :, 17, :], dv[:, :, 17, :])
            vec.tensor_add(ssv[:, :, 0], qv[:, :, 0, 0], qv[:, :, 0, 1])
            vec.tensor_add(ssv[:, :, 17], qv[:, :, 17, 0], qv[:, :, 17, 1])

            # seg_len = ss^(1/4); knot diffs are sums of seg_lens
            nc.scalar.activation(sqt[:], ss[:], AF.Sqrt, bias=zero_t[:])
            nc.scalar.activation(sl[:], sqt[:], AF.Sqrt, bias=zero_t[:])
            vec.reciprocal(rsl[:], sl[:])

            d20 = work.tile([128, J * S], F32)
            d31 = work.tile([128, J * S], F32)
            r20 = work.tile([128, J * S], F32)
            r31 = work.tile([128, J * S], F32)
            b01 = work.tile([128, J * S], F32)
            b23 = work.tile([128, J * S], F32)
            p20 = work.tile([128, J * S], F32)
            q20 = work.tile([128, J * S], F32)
            q31 = work.tile([128, J * S], F32)

            def segv(t):
                return t[:].rearrange("p (j s) -> p j s", j=J)

            vec.tensor_add(segv(d20), slv[:, :, 0:16], slv[:, :, 1:17])
            vec.tensor_add(segv(d31), slv[:, :, 1:17], slv[:, :, 2:18])
            vec.reciprocal(r20[:], d20[:])
            vec.reciprocal(r31[:], d31[:])
            vec.tensor_mul(segv(b01), slv[:, :, 1:17], rslv[:, :, 0:16])
            vec.tensor_mul(segv(b23), slv[:, :, 1:17], rslv[:, :, 2:18])
            vec.tensor_mul(segv(p20), slv[:, :, 0:16], segv(r20))
            vec.tensor_mul(segv(q20), slv[:, :, 1:17], segv(r20))
            vec.tensor_mul(segv(q31), slv[:, :, 1:17], segv(r31))

            sc = [work.tile([128, J * S], F32, name=f"sc{i}") for i in range(6)]
            b01d, b23d, dbd, ead, md, c2d = (segv(t) for t in sc)
            for d in range(D):
                dd0 = dv[:, :, 0:16, d]    # D[s]
                dd1 = dv[:, :, 1:17, d]    # D[s+1] = B12
                dd2 = dv[:, :, 2:18, d]    # D[s+2]
                g0 = cfv[:, :, :, d, 0]
                g1 = cfv[:, :, :, d, 1]
                g2 = cfv[:, :, :, d, 2]
                g3 = cfv[:, :, :, d, 3]
                vec.tensor_mul(b01d, segv(b01), dd0)
                vec.tensor_mul(b23d, segv(b23), dd2)
                vec.tensor_sub(dbd, dd1, b01d)          # dB = B12 - B01
                vec.tensor_sub(ead, b23d, dd1)          # eA' = B23 - B12
                vec.tensor_mul(md, segv(q31), ead)      # h2
                vec.tensor_mul(c2d, segv(q20), dbd)     # c2
                vec.tensor_sub(g3, md, c2d)             # g3 = h2 - c2
                vec.tensor_mul(md, segv(p20), dbd)      # reuse md as t1
                vec.tensor_add(g1, b01d, md)            # g1 = B01 + p20*dB
                vec.tensor_sub(c2d, dd1, g1)            # reuse c2d as B12 - g1
                vec.tensor_sub(g2, c2d, g3)             # g2 = B12 - g1 - g3
                vec.tensor_copy(g0, xv[:, :, :, d])     # g0 = P1 = cps[s]

            # ---- transposes: coef[b, (s,d,k)] -> coefT[(s,d,k), b] ----
            cfj = coef[:].rearrange("p (j r) -> p j r", j=J)
            lhs_tiles = []
            for j in range(J):
                tp = pst.tile([128, 128], F32)
                nc.tensor.transpose(tp[:], cfj[:, j, :], i_sb[:])
                lh = lhsp.tile([128, 128], F32, tag="lh")
                nc.scalar.copy(lh[:], tp[:])
                lhs_tiles.append(lh)

            # ---- phase C: expansion matmuls + copies + DMA out ----
            outv = out_d.ap().rearrange("(p g j) q -> p g j q", g=4, j=4)
            for g in range(4):
                osb = osbp.tile([128, 4, OUTW], F32, tag="osb")
                for jj in range(4):
                    j = 4 * g + jj
                    lh = lhs_tiles[j]
                    po0 = ps0p.tile([128, 1024], F32, tag="po0")
                    po1 = ps1p.tile([128, 1024], F32, tag="po1")
                    for c in range(4):
                        dst = (po0 if c < 2 else po1)[:, (c % 2) * 512:(c % 2 + 1) * 512]
                        if MATMUL_MODE == "rowtile":
                            nc.tensor.matmul(
                                dst,
                                lh[32 * c:32 * c + 32, :],
                                w_sb[32 * c:32 * c + 32, 512 * c:512 * (c + 1)],
                                start=True, stop=True,
                                tile_position=(32 * c, 0),
                            )
                        else:
                            nc.tensor.matmul(
                                dst, lh[:], w_sb[:, 512 * c:512 * (c + 1)],
                                start=True, stop=True,
                            )
                    nc.vector.tensor_copy(osb[:, jj, 0:1024], po0[:])
                    nc.scalar.copy(osb[:, jj, 1024:2048], po1[:])
                nc.sync.dma_start(outv[:, g, :, :], osb[:])


_NC_CACHE = {}


def _get_nc(n_reps: int = 1):
    if n_reps not in _NC_CACHE:
        _NC_CACHE[n_reps] = _build_nc(n_reps)
    return _NC_CACHE[n_reps]


def run(cps: np.ndarray, trace: bool = False, trace_cores=None):
    cps = np.ascontiguousarray(np.asarray(cps, dtype=np.float32))
    assert cps.shape == (B_FULL, S, D), cps.shape
    nc = _get_nc()
    w = _build_w()
    ident = np.eye(128, dtype=np.float32)
    flat = cps.reshape(N_CORES, BC, S * D)
    in_maps = [
        {"cps": flat[c], "wmat": w, "ident": ident} for c in range(N_CORES)
    ]
    res = run_bass_kernel_spmd(
        nc, in_maps, list(range(N_CORES)), trace=trace,
        trace_cores=trace_cores,
    )
    out = np.concatenate([res.results[c]["out"] for c in range(N_CORES)], axis=0)
    return out.reshape(B_FULL, S * P, D), res


def kernel(cps: np.ndarray) -> np.ndarray:
    out, _ = run(cps, trace=False)
    return out


# revision 11
# speedup vs baseline: 49.3926x; 49.3926x over previous
"""Trainium2 Bass kernel for ActiveSpline (centripetal Catmull-Rom spline sampling).

Full input:  cps [16384, 16, 2] f32
Full output: pts [16384, 1024, 2] f32

Math: for each batch, build the closed-curve auxiliary control points and
centripetal knot *differences* (only knot diffs appear in the Barry-Goldman
pyramid), reduce each segment's 64-point evaluation to a cubic polynomial in
the normalized parameter u in [0,1]:

    pts(u) = g0 + g1 u + g2 u^2 + g3 u^3        (per batch, segment, coord)

The g-coefficients are computed on the Vector engine (batch-major layout:
partition p holds batches 16p..16p+15), then expanded to the 64 sample points
with TensorEngine matmuls against a constant block-Vandermonde matrix:

    out[b, (s,p,d)] = sum_{(s,d,k)} coefT[(s,d,k), b] * W[(s,d,k), (s,p,d)]

Sharding: purely batch-parallel, 2048 batches per core on 8 cores.
"""

import numpy as np

import concourse.bass as bass
import concourse.bacc as bacc
import concourse.mybir as mybir
import concourse.tile as tile
from concourse.bass_utils import run_bass_kernel_spmd

F32 = mybir.dt.float32
AF = mybir.ActivationFunctionType

N_CORES = 8
B_FULL = 16384
S = 16            # control points / segments (closed curve)
D = 2
P = 64            # samples per segment
BC = B_FULL // N_CORES   # 2048 batches per core
J = 16            # batches per partition (BC = 128 * J)
EPS = 1e-7
OUTW = S * P * D  # 2048 output floats per batch

# "rowtile": 4 concurrent K=32 matmuls via tile_position (4x PE throughput for fp32)
# "fullk":   4 plain K=128 matmuls (safe fallback)
MATMUL_MODE = "rowtile"


def _build_w() -> np.ndarray:
    """W [128, 2048]: block-diagonal expansion matrix.

    Row r = 32*c + rr encodes (s = 4*c + (rr>>3), d = (rr>>2)&1, k = rr&3).
    Col n = 512*c' + nn encodes (s' = 4*c' + nn//128, p = (nn%128)//2, d' = nn&1).
    W[r, n] = (s==s' and d==d') * u_p^k.
    The diagonal 32x512 blocks are identical, so rowtile mode can slice
    W[32c:32c+32, 512c:512(c+1)] and fullk mode can slice W[:, 512c:512(c+1)].
    """
    f = np.float32
    u = (np.arange(P, dtype=f) / f(P - 1)).astype(f)
    pow_u = np.stack([np.ones(P, f), u, u * u, (u * u) * u])  # [4, 64]
    w4 = np.zeros((32, 512), f)
    for rr in range(32):
        sl, d, k = rr >> 3, (rr >> 2) & 1, rr & 3
        w4[rr, sl * 128 + np.arange(P) * 2 + d] = pow_u[k]
    w = np.zeros((128, 2048), f)
    for c in range(4):
        w[32 * c:32 * c + 32, 512 * c:512 * (c + 1)] = w4
    return w


def _build_nc(n_reps: int = 1, loop_n: int = 0):
    nc = bacc.Bacc("TRN2", target_bir_lowering=False, debug=False,
                   enable_asserts=False, num_devices=N_CORES)

    cps_d = nc.dram_tensor("cps", [BC, S * D], F32, kind="ExternalInput")
    w_d = nc.dram_tensor("wmat", [128, OUTW], F32, kind="ExternalInput")
    id_d = nc.dram_tensor("ident", [128, 128], F32, kind="ExternalInput")
    out_d = nc.dram_tensor("out", [BC, OUTW], F32, kind="ExternalOutput")

    with tile.TileContext(nc) as tc:
        with (
            tc.tile_pool(name="const", bufs=1) as const,
            tc.tile_pool(name="inp", bufs=2) as inp,
            tc.tile_pool(name="work", bufs=2) as work,
            tc.tile_pool(name="lhs", bufs=16) as lhsp,
            tc.tile_pool(name="osb", bufs=2) as osbp,
            tc.tile_pool(name="pst", bufs=2, space="PSUM") as pst,
            tc.tile_pool(name="ps0", bufs=1, space="PSUM") as ps0p,
            tc.tile_pool(name="ps1", bufs=1, space="PSUM") as ps1p,
        ):
            # ---- constants ----
            w_sb = const.tile([128, OUTW], F32)
            i_sb = const.tile([128, 128], F32)
            nc.sync.dma_start(w_sb[:], w_d.ap())
            nc.sync.dma_start(i_sb[:], id_d.ap())

            eps_t = const.tile([128, 1], F32)
            zero_t = const.tile([128, 1], F32)
            nc.vector.memset(eps_t[:], float(EPS))
            nc.vector.memset(zero_t[:], 0.0)

            if loop_n:
                with tc.For_i(0, loop_n, 1):
                    _emit_once(nc, tc, inp, work, lhsp, osbp, pst, ps0p, ps1p,
                               cps_d, out_d, w_sb, i_sb, eps_t, zero_t)
            else:
                for _rep in range(n_reps):
                    _emit_once(nc, tc, inp, work, lhsp, osbp, pst, ps0p, ps1p,
                               cps_d, out_d, w_sb, i_sb, eps_t, zero_t)

    nc.compile()
    return nc


def _emit_once(nc, tc, inp, work, lhsp, osbp, pst, ps0p, ps1p,
               cps_d, out_d, w_sb, i_sb, eps_t, zero_t):
    if True:
        if True:
            # ---- input ----
            x = inp.tile([128, J * S * D], F32)      # [p, (j, s, d)]
            nc.sync.dma_start(
                x[:].rearrange("p (j q) -> p j q", j=J),
                cps_d.ap().rearrange("(p j) q -> p j q", j=J),
            )

            xv = x[:].rearrange("p (j s d) -> p j s d", j=J, s=S, d=D)

            # ---- phase B: knots + cubic coefficients (vector engine) ----
            diff = work.tile([128, J * 18 * 2], F32)   # aux diffs, i = 0..17
            sq = work.tile([128, J * 18 * 2], F32)
            ss = work.tile([128, J * 18], F32)         # squared seg lengths
            sqt = work.tile([128, J * 18], F32)
            sl = work.tile([128, J * 18], F32)         # seg_len = ss^0.25
            rsl = work.tile([128, J * 18], F32)
            coef = work.tile([128, J * 128], F32)      # [p, (j, s, d, k)]

            dv = diff[:].rearrange("p (j i d) -> p j i d", j=J, i=18, d=D)
            qv = sq[:].rearrange("p (j i d) -> p j i d", j=J, i=18, d=D)
            ssv = ss[:].rearrange("p (j i) -> p j i", j=J)
            slv = sl[:].rearrange("p (j i) -> p j i", j=J)
            rslv = rsl[:].rearrange("p (j i) -> p j i", j=J)
            cfv = coef[:].rearrange("p (j s d k) -> p j s d k", j=J, s=S, d=D, k=4)

            vec = nc.vector
            # inner aux diffs: D[i] = cps[i] - cps[i-1] (i=1..15), D[16] = cps0 - cps15
            vec.tensor_sub(dv[:, :, 1:16, :], xv[:, :, 1:16, :], xv[:, :, 0:15, :])
            vec.tensor_sub(dv[:, :, 16, :], xv[:, :, 0, :], xv[:, :, 15, :])
            vec.tensor_mul(qv[:, :, 1:17, :], dv[:, :, 1:17, :], dv[:, :, 1:17, :])
            vec.tensor_add(ssv[:, :, 1:17], qv[:, :, 1:17, 0], qv[:, :, 1:17, 1])

            l01 = work.tile([128, J], F32)
            llast = work.tile([128, J], F32)
            rl01 = work.tile([128, J], F32)
            rllast = work.tile([128, J], F32)
            r1 = work.tile([128, J], F32)
            r2 = work.tile([128, J], F32)
            # l01/llast include the +EPS inside the sqrt (as the reference does)
            nc.scalar.activation(l01[:], ssv[:, :, 1], AF.Sqrt, bias=eps_t[:])
            nc.scalar.activation(llast[:], ssv[:, :, 16], AF.Sqrt, bias=eps_t[:])
            vec.reciprocal(rl01[:], l01[:])
            vec.reciprocal(rllast[:], llast[:])
            vec.tensor_mul(r1[:], l01[:], rllast[:])     # l01/llast
            vec.tensor_mul(r2[:], llast[:], rl01[:])     # llast/l01
            for d in range(D):
                vec.tensor_mul(dv[:, :, 0, d], r1[:], dv[:, :, 16, d])
                vec.tensor_mul(dv[:, :, 17, d], r2[:], dv[:, :, 1, d])
            vec.tensor_mul(qv[:, :, 0, :], dv[:, :, 0, :], dv[:, :, 0, :])
            vec.tensor_mul(qv[:, :, 17, :], dv[:, :, 17, :], dv[:, :, 17, :])
            vec.tensor_add(ssv[:, :, 0], qv[:, :, 0, 0], qv[:, :, 0, 1])
            vec.tensor_add(ssv[:, :, 17], qv[:, :, 17, 0], qv[:, :, 17, 1])

            # seg_len = ss^(1/4); knot diffs are sums of seg_lens
            nc.scalar.activation(sqt[:], ss[:], AF.Sqrt, bias=zero_t[:])
            nc.scalar.activation(sl[:], sqt[:], AF.Sqrt, bias=zero_t[:])
            vec.reciprocal(rsl[:], sl[:])

            d20 = work.tile([128, J * S], F32)
            d31 = work.tile([128, J * S], F32)
            r20 = work.tile([128, J * S], F32)
            r31 = work.tile([128, J * S], F32)
            b01 = work.tile([128, J * S], F32)
            b23 = work.tile([128, J * S], F32)
            p20 = work.tile([128, J * S], F32)
            q20 = work.tile([128, J * S], F32)
            q31 = work.tile([128, J * S], F32)

            def segv(t):
                return t[:].rearrange("p (j s) -> p j s", j=J)

            vec.tensor_add(segv(d20), slv[:, :, 0:16], slv[:, :, 1:17])
            vec.tensor_add(segv(d31), slv[:, :, 1:17], slv[:, :, 2:18])
            vec.reciprocal(r20[:], d20[:])
            vec.reciprocal(r31[:], d31[:])
            vec.tensor_mul(segv(b01), slv[:, :, 1:17], rslv[:, :, 0:16])
            vec.tensor_mul(segv(b23), slv[:, :, 1:17], rslv[:, :, 2:18])
            vec.tensor_mul(segv(p20), slv[:, :, 0:16], segv(r20))
            vec.tensor_mul(segv(q20), slv[:, :, 1:17], segv(r20))
            vec.tensor_mul(segv(q31), slv[:, :, 1:17], segv(r31))

            sc = [work.tile([128, J * S], F32, name=f"sc{i}") for i in range(6)]
            b01d, b23d, dbd, ead, md, c2d = (segv(t) for t in sc)
            for d in range(D):
                dd0 = dv[:, :, 0:16, d]    # D[s]
                dd1 = dv[:, :, 1:17, d]    # D[s+1] = B12
                dd2 = dv[:, :, 2:18, d]    # D[s+2]
                g0 = cfv[:, :, :, d, 0]
                g1 = cfv[:, :, :, d, 1]
                g2 = cfv[:, :, :, d, 2]
                g3 = cfv[:, :, :, d, 3]
                vec.tensor_mul(b01d, segv(b01), dd0)
                vec.tensor_mul(b23d, segv(b23), dd2)
                vec.tensor_sub(dbd, dd1, b01d)          # dB = B12 - B01
                vec.tensor_sub(ead, b23d, dd1)          # eA' = B23 - B12
                vec.tensor_mul(md, segv(q31), ead)      # h2
                vec.tensor_mul(c2d, segv(q20), dbd)     # c2
                vec.tensor_sub(g3, md, c2d)             # g3 = h2 - c2
                vec.tensor_mul(md, segv(p20), dbd)      # reuse md as t1
                vec.tensor_add(g1, b01d, md)            # g1 = B01 + p20*dB
                vec.tensor_sub(c2d, dd1, g1)            # reuse c2d as B12 - g1
                vec.tensor_sub(g2, c2d, g3)             # g2 = B12 - g1 - g3
                vec.tensor_copy(g0, xv[:, :, :, d])     # g0 = P1 = cps[s]

            # ---- transposes: coef[b, (s,d,k)] -> coefT[(s,d,k), b] ----
            cfj = coef[:].rearrange("p (j r) -> p j r", j=J)
            lhs_tiles = []
            for j in range(J):
                tp = pst.tile([128, 128], F32)
                nc.tensor.transpose(tp[:], cfj[:, j, :], i_sb[:])
                lh = lhsp.tile([128, 128], F32, tag="lh")
                nc.scalar.copy(lh[:], tp[:])
                lhs_tiles.append(lh)

            # ---- phase C: expansion matmuls + copies + DMA out ----
            outv = out_d.ap().rearrange("(p g j) q -> p g j q", g=4, j=4)
            for g in range(4):
                osb = osbp.tile([128, 4, OUTW], F32, tag="osb")
                for jj in range(4):
                    j = 4 * g + jj
                    lh = lhs_tiles[j]
                    po0 = ps0p.tile([128, 1024], F32, tag="po0")
                    po1 = ps1p.tile([128, 1024], F32, tag="po1")
                    for c in range(4):
                        dst = (po0 if c < 2 else po1)[:, (c % 2) * 512:(c % 2 + 1) * 512]
                        if MATMUL_MODE == "rowtile":
                            nc.tensor.matmul(
                                dst,
                                lh[32 * c:32 * c + 32, :],
                                w_sb[32 * c:32 * c + 32, 512 * c:512 * (c + 1)],
                                start=True, stop=True,
                                tile_position=(32 * c, 0),
                            )
                        else:
                            nc.tensor.matmul(
                                dst, lh[:], w_sb[:, 512 * c:512 * (c + 1)],
                                start=True, stop=True,
                            )
                    nc.vector.tensor_copy(osb[:, jj, 0:1024], po0[:])
                    nc.scalar.copy(osb[:, jj, 1024:2048], po1[:])
                nc.sync.dma_start(outv[:, g, :, :], osb[:])


_NC_CACHE = {}


def _get_nc(n_reps: int = 1, loop_n: int = 0):
    key = (n_reps, loop_n)
    if key not in _NC_CACHE:
        _NC_CACHE[key] = _build_nc(n_reps, loop_n)
    return _NC_CACHE[key]


def run(cps: np.ndarray, trace: bool = False, trace_cores=None):
    cps = np.ascontiguousarray(np.asarray(cps, dtype=np.float32))
    assert cps.shape == (B_FULL, S, D), cps.shape
    nc = _get_nc()
    w = _build_w()
    ident = np.eye(128, dtype=np.float32)
    flat = cps.reshape(N_CORES, BC, S * D)
    in_maps = [
        {"cps": flat[c], "wmat": w, "ident": ident} for c in range(N_CORES)
    ]
    res = run_bass_kernel_spmd(
        nc, in_maps, list(range(N_CORES)), trace=trace,
        trace_cores=trace_cores,
    )
    out = np.concatenate([res.results[c]["out"] for c in range(N_CORES)], axis=0)
    return out.reshape(B_FULL, S * P, D), res


def kernel(cps: np.ndarray) -> np.ndarray:
    out, _ = run(cps, trace=False)
    return out
